# revision 1
# baseline (speedup 1.0000x reference)
"""AttentionPooling (segment softmax + weighted segment sum) on 8 trn2 cores.

v2 strategy: shard whole segments across cores (sorted batch -> contiguous
node ranges).  Host casts x to bf16 once; nodes laid out p-outer so partition
p owns a contiguous node range and every chunk DMA is one 8KB/partition
contiguous transfer (halves HBM traffic vs fp32, no DRAM bounce).  Per
2048-node chunk: PE transposes x tiles (via identity matmul) into
channel-partitioned xt in PSUM, ACT/DVE/Pool copy them to SBUF, PE runs the
MLP matmuls (W1 stationary), ACT does tanh/exp, DVE builds onehot(segment)*e,
and a single [we | x,1] matmul per 128-node tile accumulates both the (64,256)
weighted sums and the denominators in one PSUM bank.  Softmax max-subtraction
is skipped: |s| <= ||W2||_1 + |b2| ~ 28, exp stays in fp32 range.
"""

from contextlib import ExitStack

import numpy as np
import ml_dtypes

import concourse.bass as bass
import concourse.bacc as bacc
import concourse.tile as tile
from concourse import mybir
from concourse import masks
from concourse.bass_utils import run_bass_kernel_spmd

N_CORES = 8
NUM_GRAPHS = 512
SEGS_PER_CORE = NUM_GRAPHS // N_CORES  # 64
D = 256          # in channels
H = 128          # hidden
P = 128          # partitions
TILE_N = 128     # nodes per weight tile
CHUNK_T = 16     # tiles per chunk
WAVE_T = 8       # tiles per transpose wave
CHUNK_N = TILE_N * CHUNK_T  # 2048 nodes per chunk

_BF16 = mybir.dt.bfloat16
_F32 = mybir.dt.float32
_I32 = mybir.dt.int32

# timing experiments only: stages to skip when building (results invalid)
SKIP = set()


def _build_program(n_chunks: int, b2_val: float, reps: int = 1):
    nc = bacc.Bacc()
    nmax = n_chunks * CHUNK_N
    npp = nmax // P  # nodes per partition (p-outer layout)
    assert npp == n_chunks * CHUNK_T

    x_d = nc.declare_dram_parameter("x", [nmax, D], _BF16, isOutput=False)
    bt_d = nc.declare_dram_parameter("batch_t", [P, npp + SEGS_PER_CORE], _I32, isOutput=False)
    w1_d = nc.declare_dram_parameter("w1", [D, H], _BF16, isOutput=False)
    w2_d = nc.declare_dram_parameter("w2", [H, 1], _BF16, isOutput=False)
    b1_d = nc.declare_dram_parameter("b1", [H, 1], _F32, isOutput=False)
    out_d = nc.declare_dram_parameter("out_g", [SEGS_PER_CORE, D], _F32, isOutput=True)

    # p-outer view: partition p holds nodes [p*npp, (p+1)*npp)
    x_ap = x_d[:].rearrange("(p n) ch -> p n ch", p=P)

    with tile.TileContext(nc) as tc, ExitStack() as ctx:
        const_pool = ctx.enter_context(tc.tile_pool(name="consts", bufs=1))
        xbf_pool = ctx.enter_context(tc.tile_pool(name="xbf", bufs=4))
        xt_pool = ctx.enter_context(tc.tile_pool(name="xt", bufs=3))
        h_pool = ctx.enter_context(tc.tile_pool(name="h", bufs=3))
        we_pool = ctx.enter_context(tc.tile_pool(name="we", bufs=3))
        ecol_pool = ctx.enter_context(tc.tile_pool(name="ecol", bufs=3))
        fin_pool = ctx.enter_context(tc.tile_pool(name="fin", bufs=1))
        psum_t = ctx.enter_context(
            tc.tile_pool(name="psum_t", bufs=32 // WAVE_T,
                         space=bass.MemorySpace.PSUM))
        psum_h = ctx.enter_context(
            tc.tile_pool(name="psum_h", bufs=2, space=bass.MemorySpace.PSUM))
        psum_s = ctx.enter_context(
            tc.tile_pool(name="psum_s", bufs=1, space=bass.MemorySpace.PSUM))
        psum_acc = ctx.enter_context(
            tc.tile_pool(name="psum_acc", bufs=1, space=bass.MemorySpace.PSUM))

        # ---- constants / weights ----
        # identity first (Pool engine, no DMA) so chunk-0 transposes can
        # start as soon as the first x DMA lands; weights follow on the
        # scalar ring to keep the SP ring free for x chunk loads.
        ident = const_pool.tile([P, P], _BF16, tag="ident")
        masks.make_identity(nc, ident[:])
        # bt first: cmp(0)/cmp(1) on DVE need it within ~1us, while the
        # weights aren't read until the first W1 matmuls
        bt_sb = const_pool.tile([P, npp + SEGS_PER_CORE], _I32, tag="bt")
        nc.scalar.dma_start(bt_sb[:], bt_d[:])
        iota_sb = bt_sb[:, npp:npp + SEGS_PER_CORE]
        w1_sb = const_pool.tile([P, 2, H], _BF16, tag="w1")   # [:, 0, :]=ch 0-127
        nc.scalar.dma_start(w1_sb[:, 0, :], w1_d[0:128, :])
        nc.scalar.dma_start(w1_sb[:, 1, :], w1_d[128:256, :])
        w2_sb = const_pool.tile([P, 1], _BF16, tag="w2")
        nc.scalar.dma_start(w2_sb[:], w2_d[:])
        b1_sb = const_pool.tile([P, 1], _F32, tag="b1")
        nc.scalar.dma_start(b1_sb[:], b1_d[:])

        accd_ps = psum_acc.tile([SEGS_PER_CORE, D + 1], _F32, tag="accd")

        rep_ctx = tc.For_i(0, reps, 1) if reps > 1 else None
        if rep_ctx is not None:
            rep_ctx.__enter__()

        saved = {}

        def emit_load(c):
            # contiguous bf16 load (8KB/partition) in two halves on
            # alternating rings; memset of the ones column goes FIRST so
            # DVE is not serialized behind the DMA
            x_ext = xbf_pool.tile([P, CHUNK_T, D + 1], _BF16, tag="xext")
            nc.vector.memset(x_ext[:, :, D:D + 1], 1.0)
            half_t = CHUNK_T // 2
            for i, eng in enumerate((nc.sync, nc.gpsimd)):
                t0 = c * CHUNK_T + i * half_t
                eng.dma_start(x_ext[:, i * half_t:(i + 1) * half_t, 0:D],
                              x_ap[:, t0:t0 + half_t, :])
            saved[c] = [x_ext]

        def emit_transpose(c):
            x_ext, = saved[c]
            # PE-transpose 128x128 tiles -> channel-partitioned xt
            #    4 waves of 8 tiles, each wave filling one PSUM bank
            xt = xt_pool.tile([P, 2, CHUNK_N], _BF16, tag="xt")
            waves = [(half, j0) for j0 in range(0, CHUNK_T, WAVE_T)
                     for half in (0, 1)]
            for w, (half, j0) in enumerate(
                    [] if "transpose" in SKIP else waves):
                pt = psum_t.tile([P, WAVE_T * TILE_N], _BF16, tag="pt")
                for j in range(WAVE_T):
                    nc.tensor.transpose(
                        pt[:, j * TILE_N:(j + 1) * TILE_N],
                        x_ext[:, j0 + j, half * 128:half * 128 + 128],
                        ident[:])
                dst = xt[:, half, j0 * TILE_N:(j0 + WAVE_T) * TILE_N]
                # gpsimd cannot read PSUM; alternate ACT/DVE so the last
                # copy (DVE, fastest) lands before PE needs xt for W1
                if w % 2 == 0:
                    nc.scalar.activation(dst, pt[:],
                                         mybir.ActivationFunctionType.Copy)
                else:
                    nc.vector.tensor_copy(dst, pt[:])

            # onehot compare is independent of the scores — emit it here so
            # DVE runs it early, keeping only the `we` multiply on the
            # exp -> wsum critical path
            cmp = we_pool.tile([P, CHUNK_T, SEGS_PER_CORE], _BF16, tag="cmp")
            bt_c = bt_sb[:, c * CHUNK_T:(c + 1) * CHUNK_T]
            nc.vector.tensor_tensor(
                cmp[:],
                bt_c.unsqueeze(2).broadcast_to([P, CHUNK_T, SEGS_PER_CORE]),
                iota_sb.unsqueeze(1).broadcast_to([P, CHUNK_T, SEGS_PER_CORE]),
                mybir.AluOpType.is_equal)

            saved[c] += [xt, cmp]

        def emit_score(c):
            x_ext, xt, cmp = saved[c]

            # 3+4) h = tanh(x @ W1 + b1) interleaved with per-tile score
            # matmuls (s_col[p, t] = h_tile.T @ W2) so scores start while
            # later W1 slices still stream; exp/we run in halves so the
            # next step's wsum unblocks on the first half early
            h_bf = h_pool.tile([P, CHUNK_N], _BF16, tag="h")
            ps_s = psum_s.tile([P, CHUNK_T], _F32, tag="ps_s")
            e_col = ecol_pool.tile([P, CHUNK_T], _F32, tag="ecol")
            we = we_pool.tile([P, CHUNK_T, SEGS_PER_CORE], _BF16, tag="we")
            n_sl = CHUNK_N // 512
            tiles_per_sl = CHUNK_T // n_sl

            def emit_scores(s):
                for t in range(s * tiles_per_sl, (s + 1) * tiles_per_sl):
                    nc.tensor.matmul(ps_s[:, t:t + 1],
                                     h_bf[:, t * TILE_N:(t + 1) * TILE_N],
                                     w2_sb[:], start=True, stop=True)

            def emit_we_half(i):
                th = CHUNK_T // 2
                tsl = slice(i * th, (i + 1) * th)
                nc.scalar.activation(e_col[:, tsl], ps_s[:, tsl],
                                     mybir.ActivationFunctionType.Exp,
                                     bias=float(b2_val))
                nc.vector.tensor_tensor(
                    we[:, tsl, :], cmp[:, tsl, :],
                    e_col[:, tsl].unsqueeze(2).broadcast_to(
                        [P, th, SEGS_PER_CORE]),
                    mybir.AluOpType.mult)

            for s in range(n_sl):
                ph = psum_h.tile([P, 512], _F32, tag="ph")
                sl = slice(s * 512, (s + 1) * 512)
                nc.tensor.matmul(ph[:], w1_sb[:, 0, :], xt[:, 0, sl],
                                 start=True, stop=False)
                nc.tensor.matmul(ph[:], w1_sb[:, 1, :], xt[:, 1, sl],
                                 start=False, stop=True)
                nc.scalar.activation(h_bf[:, sl], ph[:],
                                     mybir.ActivationFunctionType.Tanh,
                                     bias=b1_sb[:])
                if s >= 1:
                    emit_scores(s - 1)
                if s == n_sl // 2:
                    emit_we_half(0)
            emit_scores(n_sl - 1)
            emit_we_half(1)

            saved[c].append(we)

        def emit_wsum(c):
            x_ext, _, _, we = saved.pop(c)
            first = c == 0
            last = c == n_chunks - 1
            for t in range(CHUNK_T):
                if "wsum" in SKIP and not (c == 0 and t == 0) and not (c == n_chunks - 1 and t == CHUNK_T - 1):
                    continue
                nc.tensor.matmul(accd_ps[:], we[:, t, :], x_ext[:, t, :],
                                 start=(first and t == 0),
                                 stop=(last and t == CHUNK_T - 1),
                                 skip_group_check=True)

        # software pipeline: loads run one chunk ahead; per outer step, PE
        # sees [transposes c] [wsum c-1] [W1+score c] so the
        # transpose->copy->W1 latency hides under the wsum matmuls.
        emit_load(0)
        for c in range(n_chunks + 1):
            if c + 1 < n_chunks:
                emit_load(c + 1)
            if c < n_chunks:
                emit_transpose(c)
            if c >= 1:
                emit_wsum(c - 1)
            if c < n_chunks:
                emit_score(c)

        if rep_ctx is not None:
            rep_ctx.__exit__(None, None, None)

        # ---- epilogue: out = acc / den ----
        den_sb = fin_pool.tile([SEGS_PER_CORE, 1], _F32, tag="den_sb")
        nc.vector.tensor_scalar_add(den_sb[:], accd_ps[:, D:D + 1], 1e-30)
        rec_sb = fin_pool.tile([SEGS_PER_CORE, 1], _F32, tag="rec_sb")
        nc.vector.reciprocal(rec_sb[:], den_sb[:])
        out_sb = fin_pool.tile([SEGS_PER_CORE, D], _F32, tag="out_sb")
        nc.vector.tensor_scalar_mul(out_sb[:], accd_ps[:, 0:D], rec_sb[:])
        nc.sync.dma_start(out_d[:], out_sb[:])

    return nc


def _prepare_inputs(x, W1, b1, W2, b2, batch):
    batch = np.asarray(batch).astype(np.int64)
    # core k owns segments [64k, 64(k+1)); sorted batch -> contiguous ranges
    bounds = np.searchsorted(batch, np.arange(0, NUM_GRAPHS + 1, SEGS_PER_CORE))
    counts = np.diff(bounds)
    nmax = int(np.max(counts))
    n_chunks = max(1, (nmax + CHUNK_N - 1) // CHUNK_N)
    nmax_pad = n_chunks * CHUNK_N
    npp = nmax_pad // P

    x_bf = np.asarray(x, np.float32).astype(ml_dtypes.bfloat16)
    w1_bf = np.asarray(W1, np.float32).astype(ml_dtypes.bfloat16)
    w2_bf = np.asarray(W2, np.float32).reshape(H, 1).astype(ml_dtypes.bfloat16)
    b1_col = np.asarray(b1, np.float32).reshape(H, 1)
    iota_cols = np.tile(np.arange(SEGS_PER_CORE, dtype=np.int32), (P, 1))

    in_maps = []
    for k in range(N_CORES):
        lo, hi = int(bounds[k]), int(bounds[k + 1])
        cnt = hi - lo
        x_pad = np.zeros((nmax_pad, D), ml_dtypes.bfloat16)
        x_pad[:cnt] = x_bf[lo:hi]
        bt = np.full((nmax_pad,), -1, np.int32)
        bt[:cnt] = batch[lo:hi] - k * SEGS_PER_CORE
        bt_t = bt.reshape(P, npp)  # p-outer: row p = nodes [p*npp, (p+1)*npp)
        bt_t = np.concatenate([bt_t, iota_cols], axis=1).copy()
        in_maps.append({
            "x": x_pad,
            "batch_t": bt_t,
            "w1": w1_bf,
            "w2": w2_bf,
            "b1": b1_col,
        })
    return in_maps, n_chunks


def run(x, W1, b1, W2, b2, batch, trace=False, trace_kwargs=None, reps=1):
    in_maps, n_chunks = _prepare_inputs(x, W1, b1, W2, b2, batch)
    nc = _build_program(n_chunks, float(np.asarray(b2).reshape(-1)[0]), reps=reps)
    nc.finalize()
    res = run_bass_kernel_spmd(nc, in_maps, list(range(N_CORES)),
                               trace=trace, **(trace_kwargs or {}))
    out = np.concatenate([np.asarray(res.results[k]["out_g"], np.float32)
                          for k in range(N_CORES)], axis=0)
    return out, res


def kernel(x, W1, b1, W2, b2, batch):
    out, _ = run(x, W1, b1, W2, b2, batch)
    return out

